# revision 3
# baseline (speedup 1.0000x reference)
"""Bass kernel builder + host prep for nn_ContextualAttention on 8 trn2 cores.

Sharding: core = 2*s + q (s = sample 0..3, q = lf-half 0..1).
Window: 30 grid-row positions pi in [0,30), true row t(pi) = pi - 3 + 24q.
Score cols: [0,1440) window, [1440,1536) far_top, [1536,1632) far_bot.
Consumed (softmax/recon) cols: window pi in [2,28) -> global [96,1344), NA=1248.
S slab [128, J, NCOL] fp16 holds Sn, then S1 in place (shift sources are DMA
snapshots), then E (bf16, consumed-local cols [0,1248) = global-96, 48-col lag
margin keeps E writes behind later pass-2 reads).
"""
import numpy as np
import ml_dtypes
import contextlib
import concourse.bass as bass
from concourse import bacc, bass_isa
import concourse.tile as tile
from concourse import mybir

F16 = mybir.dt.float16
F32 = mybir.dt.float32
BF16 = mybir.dt.bfloat16
AL = mybir.AluOpType
AF = mybir.ActivationFunctionType

G = 48
J = 18
KT = 9
LB = 2304
C = 128
WINP = 30
WIN = WINP * G          # 1440
FT0 = WIN
FB0 = WIN + 96
NCOL = WIN + 192        # 1632
CONS0 = 2 * G           # 96
NA = 26 * G             # 1248
SCH = 408               # scores matmul N-chunk (4 per NCOL)
P1C = 204               # pass-1 chunk (8)
P2C = 156               # pass-2 chunk (8 per NA)
RCH = (432, 432, 384)   # recon chunks (pi-aligned)
ESC_BIAS = 1152 * 1e-4
SEGS = ((0, WIN), (FT0, FT0 + 96), (FB0, FB0 + 96))


def build(debug=False):
    nc = bacc.Bacc()
    fp_d = nc.dram_tensor("fp", [KT, 128, NCOL], F16, kind="ExternalInput")
    wt_d = nc.dram_tensor("wt", [J, KT, 128, 128], F16, kind="ExternalInput")
    bsp_d = nc.dram_tensor("bsp", [128, 2500], F32, kind="ExternalInput")
    rawt_d = nc.dram_tensor("rawt", [16, J, 128, 128], BF16, kind="ExternalInput")
    s10_d = nc.dram_tensor("s10", [128, J], F32, kind="ExternalInput")
    mbin_d = nc.dram_tensor("mbin", [128, J], F32, kind="ExternalInput")
    gate_d = nc.dram_tensor("gate", [128, 2], F32, kind="ExternalInput")
    out_d = nc.dram_tensor("out", [128, 48, 96], F32, kind="ExternalOutput")
    if debug:
        dSn_d = nc.dram_tensor("dSn", [128, J, NCOL], F16, kind="ExternalOutput")
        dS1_d = nc.dram_tensor("dS1", [128, J, NCOL], F16, kind="ExternalOutput")
        dE_d = nc.dram_tensor("dE", [128, J, NA], BF16, kind="ExternalOutput")
        dZ_d = nc.dram_tensor("dZ", [1, NA], F32, kind="ExternalOutput")
        dRd_d = nc.dram_tensor("dRd", [1, LB], F32, kind="ExternalOutput")
        dU0_d = nc.dram_tensor("dU0", [128, J, P2C], F32, kind="ExternalOutput")
        dMX0_d = nc.dram_tensor("dMX0", [128, P2C], F32, kind="ExternalOutput")
        dS20_d = nc.dram_tensor("dS20", [128, J, P2C], F32, kind="ExternalOutput")

    with tile.TileContext(nc) as tc, contextlib.ExitStack() as ctx:
        consts = ctx.enter_context(tc.tile_pool(name="consts", bufs=1))
        wtp = ctx.enter_context(tc.tile_pool(name="wtp", bufs=4))
        big = ctx.enter_context(tc.tile_pool(name="big", bufs=1))
        work = ctx.enter_context(tc.tile_pool(name="work", bufs=2))
        rows = ctx.enter_context(tc.tile_pool(name="rows", bufs=1))
        rawp = ctx.enter_context(tc.tile_pool(name="rawp", bufs=2))
        gsp = ctx.enter_context(tc.tile_pool(name="gsp", bufs=3))
        psum1 = ctx.enter_context(tc.tile_pool(name="psum1", bufs=2, space="PSUM"))

        # ---------------- consts / small inputs ----------------
        s10t = consts.tile([128, J], F32, tag="s10t")
        nc.sync.dma_start(out=s10t, in_=s10_d[:, :])
        mbint = consts.tile([128, J], F32, tag="mbint")
        nc.sync.dma_start(out=mbint, in_=mbin_d[:, :])
        gatet = consts.tile([128, 2], F32, tag="gatet")
        nc.sync.dma_start(out=gatet, in_=gate_d[:, :])
        ones32 = consts.tile([128, 1], F32, tag="ones32")
        nc.vector.memset(ones32, 1.0)
        ones16 = consts.tile([128, 1], BF16, tag="ones16")
        nc.vector.memset(ones16, 1.0)

        # ---------------- rden (row slots share the pass-2 "S2" tag) ----------------
        bspt = work.tile([128, 2500], F32, tag="S2")
        nc.sync.dma_start(out=bspt, in_=bsp_d[:, :])
        bsq = work.tile([128, 2500], F32, tag="S2")
        nc.vector.tensor_tensor(out=bsq, in0=bspt, in1=bspt, op=AL.mult)
        Qrow = rows.tile([1, 2500], F32, tag="rowA")
        with tc.tile_pool(name="psq", bufs=2, space="PSUM") as psq:
            for c0, w in ((0, 512), (512, 512), (1024, 512), (1536, 512), (2048, 452)):
                qp = psq.tile([1, 512], F32, tag="qp")
                nc.tensor.matmul(qp[:, :w], ones32, bsq[:, c0:c0 + w], start=True, stop=True)
                nc.scalar.activation(Qrow[:, c0:c0 + w], qp[:, :w], AF.Copy)
        Arow = rows.tile([1, 2500], F32, tag="rowB")
        nc.vector.tensor_tensor(out=Arow[:, 0:2499], in0=Qrow[:, 0:2499], in1=Qrow[:, 1:2500], op=AL.add)
        nc.vector.tensor_copy(Arow[:, 2499:2500], Qrow[:, 2499:2500])
        nc.vector.tensor_tensor(out=Arow[:, 1:2500], in0=Arow[:, 1:2500], in1=Qrow[:, 0:2499], op=AL.add)
        Brow = rows.tile([1, 2500], F32, tag="rowA")
        nc.vector.tensor_tensor(out=Brow[:, 0:2450], in0=Arow[:, 0:2450], in1=Arow[:, 50:2500], op=AL.add)
        nc.vector.tensor_copy(Brow[:, 2450:2500], Arow[:, 2450:2500])
        nc.vector.tensor_tensor(out=Brow[:, 50:2500], in0=Brow[:, 50:2500], in1=Arow[:, 0:2450], op=AL.add)
        ssq = rows.tile([1, LB], F32, tag="rowB")
        bview = bass.AP(tensor=Brow.tensor, offset=Brow.offset + 51,
                        ap=[Brow.ap[0], [50, 48], [1, 48]])
        nc.vector.tensor_scalar_add(ssq.rearrange("p (a b) -> p a b", a=48), bview, ESC_BIAS)
        rcp = rows.tile([1, LB], F32, tag="rowA")
        nc.vector.reciprocal(rcp, ssq)
        rdrow = rows.tile([1, LB], F32, tag="rowB")
        nc.scalar.activation(rdrow, rcp, AF.Sqrt)
        if debug:
            nc.sync.dma_start(out=dRd_d[:, :], in_=rdrow)
        rdent = consts.tile([128, J], F32, tag="rdent")
        rd_scr = nc.dram_tensor("rd_scratch", [LB], F32)
        nc.sync.dma_start(out=rd_scr[:], in_=rdrow)
        rview = bass.AP(tensor=rd_scr.ap().tensor, offset=0, ap=[[1, 128], [128, J]])
        nc.sync.dma_start(out=rdent, in_=rview)

        # ---------------- scores GEMM + normalized drain into slab ----------------
        fpt = big.tile([128, KT, NCOL], F16, tag="U1")
        for o in range(KT):
            nc.sync.dma_start(out=fpt[:, o, :], in_=fp_d[o, :, :])
        slab = big.tile([128, J, NCOL], F16, tag="slab")   # Sn -> S1 in place
        with tc.tile_pool(name="psc", bufs=2, space="PSUM") as psc:
            for j in range(J):
                for pair in range(2):
                    ps = psc.tile([128, 2, 512], F32, tag="sps")
                    for o in range(KT):
                        wb = wtp.tile([128, 128], F16, tag="wb")
                        nc.sync.dma_start(out=wb, in_=wt_d[j, o, :, :])
                        for h in range(2):
                            ch = pair * 2 + h
                            nc.tensor.matmul(ps[:, h, 0:SCH], wb,
                                             fpt[:, o, ch * SCH:(ch + 1) * SCH],
                                             start=(o == 0), stop=(o == KT - 1))
                    for h in range(2):
                        ch = pair * 2 + h
                        nc.scalar.activation(slab[:, j, ch * SCH:(ch + 1) * SCH],
                                             ps[:, h, 0:SCH], AF.Copy,
                                             scale=rdent[:, j:j + 1])
        if debug:
            nc.sync.dma_start(out=dSn_d[:, :, :], in_=slab)

        # ---------------- pass-1 in place: S1 = Sn + diag(+1) + diag(-1) ----------
        SnM_prev = None
        for ci in range(NCOL // P1C):
            c0 = ci * P1C
            SnP = work.tile([128, J, P1C + 1], F16, tag="shA")
            SnM = work.tile([128, J, P1C + 1], F16, tag="shB")
            # zero garbage rows: SnP row (127, J-1) = lb 2303 + 1 (none), SnM (0,0) = lb0 - 1
            nc.gpsimd.memset(SnP[96:128, J - 1:J, :], 0.0)
            nc.gpsimd.memset(SnM[0:1, 0:1, :], 0.0)
            wP = min(P1C + 1, NCOL - c0)
            nc.sync.dma_start(out=SnP[0:127, :, 0:wP], in_=slab[1:128, :, c0:c0 + wP])
            nc.sync.dma_start(out=SnP[127:128, 0:J - 1, 0:wP], in_=slab[0:1, 1:J, c0:c0 + wP])
            if wP < P1C + 1:
                nc.gpsimd.memset(SnP[:, :, wP:P1C + 1], 0.0)
            # SnM local i in [1,205) <- slab[rows-1, c0 .. c0+204); local col 0 chained
            nc.sync.dma_start(out=SnM[1:128, :, 1:P1C + 1], in_=slab[0:127, :, c0:c0 + P1C])
            nc.sync.dma_start(out=SnM[0:1, 1:J, 1:P1C + 1], in_=slab[127:128, 0:J - 1, c0:c0 + P1C])
            if ci == 0:
                nc.gpsimd.memset(SnM[:, :, 0:1], 0.0)
            else:
                nc.sync.dma_start(out=SnM[:, :, 0:1], in_=SnM_prev[:, :, P1C:P1C + 1])
            SnM_prev = SnM
            for slo, shi in SEGS:
                a = max(c0, slo)
                b = min(c0 + P1C, shi)
                if a >= b:
                    continue
                e = min(b, shi - 1)
                if a < e:
                    nc.vector.tensor_tensor(out=slab[:, :, a:e], in0=slab[:, :, a:e],
                                            in1=SnP[:, :, a - c0 + 1:e - c0 + 1], op=AL.add)
                a2 = max(a, slo + 1)
                if a2 < b:
                    nc.vector.tensor_tensor(out=slab[:, :, a2:b], in0=slab[:, :, a2:b],
                                            in1=SnM[:, :, a2 - c0:b - c0], op=AL.add)
        # gates: zero pi=2 block (q=0) / pi=27 block (q=1)
        nc.vector.tensor_scalar_mul(slab[:, :, 2 * G:3 * G], slab[:, :, 2 * G:3 * G], gatet[:, 0:1])
        nc.vector.tensor_scalar_mul(slab[:, :, 27 * G:28 * G], slab[:, :, 27 * G:28 * G], gatet[:, 1:2])
        if debug:
            nc.sync.dma_start(out=dS1_d[:, :, :], in_=slab)

        # far-correction shifted sources (snapshots, small)
        ftP = consts.tile([128, J, 96], F16, tag="ftP")
        nc.gpsimd.memset(ftP[64:128, J - 1:J, :], 0.0)
        nc.sync.dma_start(out=ftP[0:80, :, :], in_=slab[48:128, :, FT0:FT0 + 96])
        nc.sync.dma_start(out=ftP[80:128, 0:J - 1, :], in_=slab[0:48, 1:J, FT0:FT0 + 96])
        fbM = consts.tile([128, J, 96], F16, tag="fbM")
        nc.gpsimd.memset(fbM[0:64, 0:1, :], 0.0)
        nc.sync.dma_start(out=fbM[48:128, :, :], in_=slab[0:80, :, FB0:FB0 + 96])
        nc.sync.dma_start(out=fbM[0:48, 1:J, :], in_=slab[80:128, 0:J - 1, FB0:FB0 + 96])
        addC = consts.tile([128, 96], F16, tag="addC")
        nc.gpsimd.memset(addC, 0.0)
        nc.sync.dma_start(out=addC[80:127, :], in_=slab[1:48, 0, FT0:FT0 + 96])
        addCp = consts.tile([128, 96], F16, tag="addCp")
        nc.gpsimd.memset(addCp, 0.0)
        nc.sync.dma_start(out=addCp[1:48, :], in_=slab[80:127, 17, FB0:FB0 + 96])

        # E overlays slab cols [0, NA) as bf16 (48-col lag margin vs pass-2 reads)
        Ebig = slab[:, :, 0:NA].bitcast(BF16)
        Zrow = consts.tile([1, NA], F32, tag="Zrow")

        # ---------------- pass-2 + softmax per chunk ----------------
        for ci in range(NA // P2C):
            d0 = ci * P2C            # consumed-local col
            g0 = CONS0 + d0          # global col
            S1P = work.tile([128, J, P2C], F16, tag="shA")
            nc.gpsimd.memset(S1P[64:128, J - 1:J, :], 0.0)
            nc.sync.dma_start(out=S1P[0:80, :, :], in_=slab[48:128, :, g0 + 48:g0 + 48 + P2C])
            nc.sync.dma_start(out=S1P[80:128, 0:J - 1, :], in_=slab[0:48, 1:J, g0 + 48:g0 + 48 + P2C])
            S1M = work.tile([128, J, P2C], F16, tag="shB")
            nc.gpsimd.memset(S1M[0:64, 0:1, :], 0.0)
            nc.sync.dma_start(out=S1M[48:128, :, :], in_=slab[0:80, :, g0 - 48:g0 - 48 + P2C])
            nc.sync.dma_start(out=S1M[0:48, 1:J, :], in_=slab[80:128, 0:J - 1, g0 - 48:g0 - 48 + P2C])
            adA = work.tile([128, P2C], F16, tag="adA")
            nc.gpsimd.memset(adA, 0.0)
            nc.sync.dma_start(out=adA[80:127, :], in_=slab[1:48, 0, g0 + 48:g0 + 48 + P2C])
            adAp = work.tile([128, P2C], F16, tag="adAp")
            nc.gpsimd.memset(adAp, 0.0)
            nc.sync.dma_start(out=adAp[1:48, :], in_=slab[80:127, 17, g0 - 48:g0 - 48 + P2C])
            S2 = work.tile([128, J, P2C], F32, tag="S2")
            nc.vector.tensor_tensor(out=S2, in0=slab[:, :, g0:g0 + P2C], in1=S1P, op=AL.add)
            nc.vector.tensor_tensor(out=S2, in0=S2, in1=S1M, op=AL.add)
            nc.vector.tensor_tensor(out=S2[:, 17, :], in0=S2[:, 17, :], in1=adA, op=AL.add)
            nc.vector.tensor_tensor(out=S2[:, 0, :], in0=S2[:, 0, :], in1=adAp, op=AL.add)
            # far corrections
            b_lo, b_hi = 26 * G, 26 * G + G - 1          # global target cols (B/C)
            if b_lo < g0 + P2C and b_hi > g0:
                a = max(b_lo, g0); b = min(b_hi, g0 + P2C)
                so = a - b_lo
                nc.vector.tensor_tensor(out=S2[:, :, a - g0:b - g0], in0=S2[:, :, a - g0:b - g0],
                                        in1=ftP[:, :, 1 + so:1 + so + (b - a)], op=AL.add)
                nc.vector.tensor_tensor(out=S2[:, 17, a - g0:b - g0], in0=S2[:, 17, a - g0:b - g0],
                                        in1=addC[:, 1 + so:1 + so + (b - a)], op=AL.add)
            bp_lo, bp_hi = 3 * G + 1, 3 * G + G          # global target cols (B'/C')
            if bp_lo < g0 + P2C and bp_hi > g0:
                a = max(bp_lo, g0); b = min(bp_hi, g0 + P2C)
                so = a - bp_lo
                nc.vector.tensor_tensor(out=S2[:, :, a - g0:b - g0], in0=S2[:, :, a - g0:b - g0],
                                        in1=fbM[:, :, 48 + so:48 + so + (b - a)], op=AL.add)
                nc.vector.tensor_tensor(out=S2[:, 0, a - g0:b - g0], in0=S2[:, 0, a - g0:b - g0],
                                        in1=addCp[:, 48 + so:48 + so + (b - a)], op=AL.add)
            # L = S2 * s10 per tile (in place)
            for j in range(J):
                nc.vector.tensor_scalar_mul(S2[:, j, :], S2[:, j, :], s10t[:, j:j + 1])
            # max over lb
            t9 = work.tile([128, 9, P2C], F32, tag="shA")
            nc.vector.tensor_tensor(out=t9, in0=S2[:, 0:9, :], in1=S2[:, 9:18, :], op=AL.max)
            t4 = work.tile([128, 4, P2C], F32, tag="shB")
            nc.vector.tensor_tensor(out=t4, in0=t9[:, 0:4, :], in1=t9[:, 4:8, :], op=AL.max)
            t2 = work.tile([128, 2, P2C], F32, tag="t2")
            nc.vector.tensor_tensor(out=t2, in0=t4[:, 0:2, :], in1=t4[:, 2:4, :], op=AL.max)
            mx = work.tile([128, P2C], F32, tag="mx")
            nc.vector.tensor_tensor(out=mx, in0=t2[:, 0, :], in1=t2[:, 1, :], op=AL.max)
            nc.vector.tensor_tensor(out=mx, in0=mx, in1=t9[:, 8, :], op=AL.max)
            mxb = work.tile([128, P2C], F32, tag="mxb")
            nc.gpsimd.partition_all_reduce(mxb, mx, channels=128, reduce_op=bass_isa.ReduceOp.max)
            if debug and ci == 0:
                nc.sync.dma_start(out=dS20_d[:, :, :], in_=S2)
                nc.sync.dma_start(out=dMX0_d[:, :], in_=mxb)
            mview = bass.AP(tensor=mxb.tensor, offset=mxb.offset,
                            ap=[mxb.ap[0], [0, J], mxb.ap[1]])
            nc.vector.tensor_tensor(out=S2, in0=S2, in1=mview, op=AL.subtract)
            if debug and ci == 0:
                nc.sync.dma_start(out=dU0_d[:, :, :], in_=S2)
            # E = exp(u) -> bf16 overlay
            nc.scalar.activation(Ebig[:, :, d0:d0 + P2C], S2, AF.Exp)
            # Z = ones^T E
            zp = psum1.tile([1, P2C], F32, tag="zp")
            for j in range(J):
                nc.tensor.matmul(zp, ones16, Ebig[:, j, d0:d0 + P2C],
                                 start=(j == 0), stop=(j == J - 1))
            nc.scalar.activation(Zrow[:, d0:d0 + P2C], zp, AF.Copy)
            # zero masked rows for recon
            for j in range(J):
                nc.vector.tensor_scalar_mul(Ebig[:, j, d0:d0 + P2C], Ebig[:, j, d0:d0 + P2C],
                                            mbint[:, j:j + 1])
        # E gates (phantom att cols)
        nc.vector.tensor_scalar_mul(Ebig[:, :, 0:G], Ebig[:, :, 0:G], gatet[:, 0:1])
        nc.vector.tensor_scalar_mul(Ebig[:, :, NA - G:NA], Ebig[:, :, NA - G:NA], gatet[:, 1:2])
        if debug:
            nc.sync.dma_start(out=dE_d[:, :, :], in_=Ebig)
            nc.sync.dma_start(out=dZ_d[:, :], in_=Zrow)

        # recipZ = 0.25 / Z broadcast
        rzrow = consts.tile([1, NA], F32, tag="rzrow")
        nc.vector.reciprocal(rzrow, Zrow)
        nc.vector.tensor_scalar_mul(rzrow, rzrow, 0.25)
        rzb = consts.tile([128, NA], F32, tag="rzb")
        nc.gpsimd.partition_broadcast(rzb, rzrow)

        # ---------------- recon + interleave ----------------
        out_acc = big.tile([128, 48, 96], F32, tag="U1")
        nc.vector.memset(out_acc, 0.0)
        ky_pis = {0: (4, 28), 1: (3, 27), 2: (3, 27), 3: (2, 26)}
        kx_us = {0: (1, 48), 1: (0, 48), 2: (0, 48), 3: (0, 47)}
        rchunk_off = [0, 432, 864]
        with tc.tile_pool(name="psg", bufs=4, space="PSUM") as psg:
            for tap in range(16):
                ky, kx = tap // 4, tap % 4
                rawtile = rawp.tile([128, J, 128], BF16, tag="rawtile")
                rin = bass.AP(tensor=rawt_d.ap().tensor, offset=tap * J * 128 * 128,
                              ap=[[128, 128], [128 * 128, J], [1, 128]])
                nc.sync.dma_start(out=rawtile, in_=rin)
                for ri, rw in enumerate(RCH):
                    r0 = rchunk_off[ri]
                    gp = psg.tile([128, 512], F32, tag="gp")
                    for j in range(J):
                        nc.tensor.matmul(gp[:, 0:rw], rawtile[:, j, :], Ebig[:, j, r0:r0 + rw],
                                         start=(j == 0), stop=(j == J - 1))
                    gs = gsp.tile([128, 512], BF16, tag="gs")
                    nc.vector.tensor_tensor(out=gs[:, 0:rw], in0=gp[:, 0:rw],
                                            in1=rzb[:, r0:r0 + rw], op=AL.mult)
                    plo, phi = ky_pis[ky]
                    ulo, uhi = kx_us[kx]
                    cplo = 2 + r0 // G
                    cphi = 2 + (r0 + rw) // G
                    a = max(plo, cplo); bnd = min(phi, cphi)
                    if a >= bnd:
                        continue
                    npi = bnd - a
                    nu = uhi - ulo
                    goff = (a - 2) * G + ulo - r0
                    gview = bass.AP(tensor=gs.tensor, offset=gs.offset + goff,
                                    ap=[gs.ap[0], [G, npi], [1, nu]])
                    yl0 = 2 * (a - 3) + ky - 1
                    xl0 = 2 * ulo + kx - 1
                    oview = bass.AP(tensor=out_acc.tensor, offset=out_acc.offset + yl0 * 96 + xl0,
                                    ap=[out_acc.ap[0], [192, npi], [2, nu]])
                    nc.gpsimd.tensor_tensor(out=oview, in0=oview, in1=gview, op=AL.add)
        nc.sync.dma_start(out=out_d[:, :, :], in_=out_acc)
    nc.finalize()
    return nc


# ======================= host side =======================

def prep_core_inputs(f, b, mask):
    """Full inputs -> list of 8 in_map dicts (core = 2*s + q)."""
    B = f.shape[0]
    ms = np.pad(mask[0][:, ::8, ::8][0], 1)
    w = np.lib.stride_tricks.sliding_window_view(ms, (3, 3))
    mm = (w.sum((2, 3)) == 0).astype(np.float32).reshape(LB)
    s10 = np.ascontiguousarray((10.0 * mm).reshape(J, 128).T)
    mbin = np.ascontiguousarray(mm.reshape(J, 128).T)
    in_maps = []
    for s in range(B):
        fs = f[s][:, ::2, ::2]
        bs = b[s][:, ::2, ::2]
        fsp = np.pad(fs, ((0, 0), (1, 1), (1, 1)))
        bsp = np.pad(bs, ((0, 0), (1, 1), (1, 1)))
        bhwc = np.pad(b[s], ((0, 0), (1, 1), (1, 1))).transpose(1, 2, 0)
        wt = np.empty((KT, C, LB), np.float32)
        for o in range(KT):
            dy, dx = o // 3, o % 3
            wt[o] = bsp[:, dy:dy + G, dx:dx + G].reshape(C, LB)
        wt_blocks = np.ascontiguousarray(
            wt.reshape(KT, C, J, 128).transpose(2, 0, 1, 3)).astype(np.float16)
        iy, ix = np.divmod(np.arange(LB), G)
        rawt = np.empty((16, LB, C), np.float32)
        for ky in range(4):
            for kx in range(4):
                rawt[ky * 4 + kx] = bhwc[2 * iy + ky, 2 * ix + kx, :]
        rawt = np.ascontiguousarray(rawt.reshape(16, J, 128, C)).astype(ml_dtypes.bfloat16)
        bspf = np.ascontiguousarray(bsp.reshape(C, 2500))
        for q in (0, 1):
            ts_ = np.arange(WINP) - 3 + 24 * q
            fcols = np.zeros((KT, C, NCOL), np.float32)
            valid = (ts_ >= 0) & (ts_ < G)
            for o in range(KT):
                dy, dx = o // 3, o % 3
                block = fsp[:, (ts_ + dy).clip(0, G + 1), :][:, :, dx:dx + G]
                block = block * valid[None, :, None]
                fcols[o, :, :WIN] = block.reshape(C, WIN)
                if q == 1:
                    fcols[o, :, FT0:FT0 + 96] = fsp[:, dy:dy + 2, dx:dx + G].reshape(C, 96)
                else:
                    fcols[o, :, FB0:FB0 + 96] = fsp[:, 46 + dy:48 + dy, dx:dx + G].reshape(C, 96)
            gate = np.zeros((128, 2), np.float32)
            gate[:, 0] = 0.0 if q == 0 else 1.0
            gate[:, 1] = 1.0 if q == 0 else 0.0
            in_maps.append(dict(
                fp=fcols.astype(np.float16),
                wt=wt_blocks,
                bsp=bspf,
                rawt=rawt,
                s10=s10, mbin=mbin, gate=gate,
            ))
    return in_maps


def assemble(results, B=4):
    out = np.zeros((B, C, 96, 96), np.float32)
    for s in range(B):
        for q in (0, 1):
            out[s, :, 48 * q:48 * q + 48, :] = results[2 * s + q]["out"]
    return out


# ======================= self-contained runner =======================
# kernel(**inputs) entry point: full inputs in, full output out.
_NC_CACHE = {}
last_exec_time_ns = None
last_result = None


def kernel(f, b, mask):
    global last_exec_time_ns
    import os
    from concourse.bass_utils import run_bass_kernel_spmd
    f = np.ascontiguousarray(np.asarray(f, dtype=np.float32))
    b = np.ascontiguousarray(np.asarray(b, dtype=np.float32))
    mask = np.ascontiguousarray(np.asarray(mask, dtype=np.float32))
    in_maps = prep_core_inputs(f, b, mask)
    if "nc" not in _NC_CACHE:
        _NC_CACHE["nc"] = build(debug=False)
    nc = _NC_CACHE["nc"]
    trace = bool(os.environ.get("BASS_TRACE"))
    res = run_bass_kernel_spmd(nc, in_maps, core_ids=list(range(8)), trace=trace)
    global last_result
    last_result = res
    last_exec_time_ns = res.exec_time_ns
    return assemble([res.results[i] for i in range(8)], B=f.shape[0])



# revision 9
# speedup vs baseline: 2.0030x; 2.0030x over previous
"""Bass kernel builder + host prep for nn_ContextualAttention on 8 trn2 cores.

Sharding: core = 2*s + q (s = sample 0..3, q = lf-half 0..1).
Window: 30 grid-row positions pi in [0,30), true row t(pi) = pi - 3 + 24q.
Score cols: [0,1440) window, [1440,1536) far_top, [1536,1632) far_bot.
Consumed (softmax/recon) cols: window pi in [2,28) -> global [96,1344), NA=1248.

v2: no SBUF->SBUF shift DMAs. All partition shifts (diagonal fuse pass-1/2,
far corrections) are tensor-engine permutation matmuls into PSUM; hosts ships
shift matrices. slab1 holds Sn (f16, 1632 cols, E bf16 overlays cols [0,1248)
after pass-1); slab2 holds S1 for exactly the consumed sources: cols [0,1344)
= global [48,1392), [1344,1392) = far_top [1441,1488), [1392,1440) = far_bot
[1584,1631). rden moved to host (rdent input). wt/rawt layouts are dense per
partition.
"""
import numpy as np
import ml_dtypes
import contextlib
import concourse.bass as bass
from concourse import bacc, bass_isa
import concourse.tile as tile
from concourse import mybir

F16 = mybir.dt.float16
F32 = mybir.dt.float32
BF16 = mybir.dt.bfloat16
AL = mybir.AluOpType
AF = mybir.ActivationFunctionType

G = 48
J = 18
KT = 9
LB = 2304
C = 128
WINP = 30
WIN = WINP * G          # 1440
FT0 = WIN               # 1440
FB0 = WIN + 96          # 1536
NCOL = WIN + 192        # 1632
NA = 26 * G             # 1248
ESC_BIAS = 1152 * 1e-4

# shift-matrix indices in shm input
UP1, DN1, CUP1, CDN1, UP48, DN48, CUP48, CDN48, P79, PM79 = range(10)

# scores GEMM chunks over slab1 cols
SCORE_CHUNKS = ((0, 512), (512, 512), (1024, 512), (1536, 96))
# pass-1 chunks over slab2 cols (far sub-blocks only need 47 cols each;
# slab2 cols 1391 and 1439 stay unwritten/unread)
P1_CHUNKS = ((0, 512), (512, 512), (1024, 320), (1344, 47), (1392, 47))
# pass-2 chunks: (global col start, width); local d0 = g0 - 96
P2_CHUNKS = ((96, 432), (528, 432), (960, 384))
# far correction target global col ranges (47 wide each)
B_LO, B_HI = 26 * G, 26 * G + G - 1        # ftP/addC targets (chunk 2)
BP_LO, BP_HI = 3 * G + 1, 3 * G + G        # fbM/addCp targets (chunk 0)


def s2g(c):
    """slab2 col -> slab1 (global score) col."""
    if c < 1344:
        return c + 48
    if c < 1392:
        return c - 1344 + 1441
    return c - 1392 + 1584


def build(debug=False):
    nc = bacc.Bacc()
    fp_d = nc.dram_tensor("fp", [KT, 128, NCOL], F16, kind="ExternalInput")
    wt_d = nc.dram_tensor("wt", [J, 128, KT * 128], F16, kind="ExternalInput")
    rawt_d = nc.dram_tensor("rawt", [16, 128, J * 128], BF16, kind="ExternalInput")
    shm_d = nc.dram_tensor("shm", [10, 128, 128], F16, kind="ExternalInput")
    rdent_d = nc.dram_tensor("rdent", [128, J], F32, kind="ExternalInput")
    s10_d = nc.dram_tensor("s10", [128, J], F32, kind="ExternalInput")
    mbin_d = nc.dram_tensor("mbin", [128, J], F32, kind="ExternalInput")
    gate_d = nc.dram_tensor("gate", [128, 2], F32, kind="ExternalInput")
    out_d = nc.dram_tensor("out", [128, 48, 96], F32, kind="ExternalOutput")
    if debug:
        dSn_d = nc.dram_tensor("dSn", [128, J, NCOL], F16, kind="ExternalOutput")
        dS1_d = nc.dram_tensor("dS1", [128, J, 1440], F16, kind="ExternalOutput")
        dE_d = nc.dram_tensor("dE", [128, J, NA], BF16, kind="ExternalOutput")
        dZ_d = nc.dram_tensor("dZ", [1, NA], F32, kind="ExternalOutput")

    with tile.TileContext(nc) as tc, contextlib.ExitStack() as ctx:
        consts = ctx.enter_context(tc.tile_pool(name="consts", bufs=1))
        wtp = ctx.enter_context(tc.tile_pool(name="wtp", bufs=2))
        big = ctx.enter_context(tc.tile_pool(name="big", bufs=1))
        lpool = ctx.enter_context(tc.tile_pool(name="lpool", bufs=1))
        work = ctx.enter_context(tc.tile_pool(name="work", bufs=1))
        rawp = ctx.enter_context(tc.tile_pool(name="rawp", bufs=2))
        gsp = ctx.enter_context(tc.tile_pool(name="gsp", bufs=2))

        # ---------------- consts / small inputs ----------------
        s10t = consts.tile([128, J], F32, tag="s10t")
        nc.sync.dma_start(out=s10t, in_=s10_d[:, :])
        mbint = consts.tile([128, J], F32, tag="mbint")
        nc.sync.dma_start(out=mbint, in_=mbin_d[:, :])
        gatet = consts.tile([128, 2], F32, tag="gatet")
        nc.sync.dma_start(out=gatet, in_=gate_d[:, :])
        rdent = consts.tile([128, J], F32, tag="rdent")
        nc.sync.dma_start(out=rdent, in_=rdent_d[:, :])
        shmt = consts.tile([128, 10, 128], F16, tag="shmt")
        for i in range(10):
            nc.sync.dma_start(out=shmt[:, i, :], in_=shm_d[i, :, :])
        ones16 = consts.tile([128, 1], BF16, tag="ones16")
        nc.vector.memset(ones16, 1.0)

        def shmat(i):
            return shmt[:, i, :]

        # ---------------- scores GEMM -> slab1 (Sn, f16) ----------------
        fpt = big.tile([128, KT, NCOL], F16, tag="U1")
        for o in range(KT):
            nc.sync.dma_start(out=fpt[:, o, :], in_=fp_d[o, :, :])
        slab1 = big.tile([128, J, NCOL], F16, tag="slab1")
        with tc.tile_pool(name="psc", bufs=4, space="PSUM") as psc:
            for j in range(J):
                wtj = wtp.tile([128, KT * 128], F16, tag="wtj")
                nc.sync.dma_start(out=wtj, in_=wt_d[j, :, :])
                for c0, w in SCORE_CHUNKS:
                    ps = psc.tile([128, 512], F32, tag="sps")
                    for o in range(KT):
                        nc.tensor.matmul(ps[:, 0:w], wtj[:, o * 128:(o + 1) * 128],
                                         fpt[:, o, c0:c0 + w],
                                         start=(o == 0), stop=(o == KT - 1))
                    nc.scalar.activation(slab1[:, j, c0:c0 + w], ps[:, 0:w],
                                         AF.Copy, scale=rdent[:, j:j + 1])
        if debug:
            nc.sync.dma_start(out=dSn_d[:, :, :], in_=slab1)

        # ---------------- pass-1: S1 = Sn + diag(+1) + diag(-1) -> slab2 ----
        # slab2 shares the U1 slot with fpt (dead after scores GEMM) and
        # out_acc (recon starts after pass-2 ends)
        slab2 = big.tile([128, J, 1440], F16, tag="U1")
        with tc.tile_pool(name="psp1", bufs=4, space="PSUM") as psp1:
            for c0, w in P1_CHUNKS:
                g0 = s2g(c0)
                for j in range(J):
                    ps = psp1.tile([128, 512], F32, tag="p1ps")
                    nc.tensor.matmul(ps[:, 0:w], shmat(UP1),
                                     slab1[:, j, g0 + 1:g0 + 1 + w],
                                     start=True, stop=False)
                    if j < J - 1:
                        nc.tensor.matmul(ps[:, 0:w], shmat(CUP1),
                                         slab1[:, j + 1, g0 + 1:g0 + 1 + w],
                                         start=False, stop=False)
                    if j > 0:
                        nc.tensor.matmul(ps[:, 0:w], shmat(CDN1),
                                         slab1[:, j - 1, g0 - 1:g0 - 1 + w],
                                         start=False, stop=False)
                    nc.tensor.matmul(ps[:, 0:w], shmat(DN1),
                                     slab1[:, j, g0 - 1:g0 - 1 + w],
                                     start=False, stop=True)
                    nc.vector.tensor_tensor(out=slab2[:, j, c0:c0 + w],
                                            in0=ps[:, 0:w],
                                            in1=slab1[:, j, g0:g0 + w], op=AL.add)
        # gates: zero pi=2 block (q=0) / pi=27 block (q=1); slab2 cols = g-48
        nc.vector.tensor_scalar_mul(slab2[:, :, 48:96], slab2[:, :, 48:96],
                                    gatet[:, 0:1])
        nc.vector.tensor_scalar_mul(slab2[:, :, 1248:1296], slab2[:, :, 1248:1296],
                                    gatet[:, 1:2])
        if debug:
            nc.sync.dma_start(out=dS1_d[:, :, :], in_=slab2)

        # E overlays slab1 cols [0, NA) as bf16 (Sn dead after pass-1)
        Ebig = slab1[:, :, 0:NA].bitcast(BF16)
        Zrow = consts.tile([1, NA], F32, tag="Zrow")

        # ---------------- pass-2 + softmax per chunk ----------------
        with tc.tile_pool(name="psp2", bufs=4, space="PSUM") as psp2, \
             tc.tile_pool(name="psz", bufs=2, space="PSUM") as psz:
            for ci, (g0, w) in enumerate(P2_CHUNKS):
                d0 = g0 - 96
                c0 = g0 - 48                       # slab2 col of g0
                Lt = lpool.tile([128, J, 432], F32, tag="Lt")
                for j in range(J):
                    ps = psp2.tile([128, 432], F32, tag="p2ps")
                    nc.tensor.matmul(ps[:, 0:w], shmat(UP48),
                                     slab2[:, j, c0 + 48:c0 + 48 + w],
                                     start=True, stop=False)
                    if j < J - 1:
                        nc.tensor.matmul(ps[:, 0:w], shmat(CUP48),
                                         slab2[:, j + 1, c0 + 48:c0 + 48 + w],
                                         start=False, stop=False)
                    if j > 0:
                        nc.tensor.matmul(ps[:, 0:w], shmat(CDN48),
                                         slab2[:, j - 1, c0 - 48:c0 - 48 + w],
                                         start=False, stop=False)
                    # row-wrap terms (by=47 up-wrap at j=17, by=0 dn-wrap at j=0)
                    if j == J - 1:
                        nc.tensor.matmul(ps[:, 0:w], shmat(P79),
                                         slab2[:, 0, c0 + 48:c0 + 48 + w],
                                         start=False, stop=False)
                    if j == 0:
                        nc.tensor.matmul(ps[:, 0:w], shmat(PM79),
                                         slab2[:, J - 1, c0 - 48:c0 - 48 + w],
                                         start=False, stop=False)
                    if ci == 2:
                        # B targets [B_LO, B_HI): psum cols, ft sources
                        a, b = B_LO - g0, B_HI - g0
                        nw = b - a
                        nc.tensor.matmul(ps[:, a:b], shmat(UP48),
                                         slab2[:, j, 1344:1344 + nw],
                                         start=False, stop=False)
                        if j < J - 1:
                            nc.tensor.matmul(ps[:, a:b], shmat(CUP48),
                                             slab2[:, j + 1, 1344:1344 + nw],
                                             start=False, stop=False)
                        if j == J - 1:
                            nc.tensor.matmul(ps[:, a:b], shmat(P79),
                                             slab2[:, 0, 1344:1344 + nw],
                                             start=False, stop=False)
                    if ci == 0:
                        # B' targets [BP_LO, BP_HI): fb sources
                        a, b = BP_LO - g0, BP_HI - g0
                        nw = b - a
                        nc.tensor.matmul(ps[:, a:b], shmat(DN48),
                                         slab2[:, j, 1392:1392 + nw],
                                         start=False, stop=False)
                        if j > 0:
                            nc.tensor.matmul(ps[:, a:b], shmat(CDN48),
                                             slab2[:, j - 1, 1392:1392 + nw],
                                             start=False, stop=False)
                        if j == 0:
                            nc.tensor.matmul(ps[:, a:b], shmat(PM79),
                                             slab2[:, J - 1, 1392:1392 + nw],
                                             start=False, stop=False)
                    nc.tensor.matmul(ps[:, 0:w], shmat(DN48),
                                     slab2[:, j, c0 - 48:c0 - 48 + w],
                                     start=False, stop=True)
                    # S2 = psum + S1, then L = S2 * s10 (scalar engine)
                    nc.vector.tensor_tensor(out=Lt[:, j, 0:w], in0=ps[:, 0:w],
                                            in1=slab2[:, j, c0:c0 + w], op=AL.add)
                    nc.scalar.activation(Lt[:, j, 0:w], Lt[:, j, 0:w],
                                         AF.Copy, scale=s10t[:, j:j + 1])
                # max over lb: tree over j, then across partitions
                t9 = work.tile([128, 9, 432], F32, tag="shA")
                nc.vector.tensor_tensor(out=t9[:, :, 0:w], in0=Lt[:, 0:9, 0:w],
                                        in1=Lt[:, 9:18, 0:w], op=AL.max)
                t4 = work.tile([128, 4, 432], F32, tag="shB")
                nc.vector.tensor_tensor(out=t4[:, :, 0:w], in0=t9[:, 0:4, 0:w],
                                        in1=t9[:, 4:8, 0:w], op=AL.max)
                t2 = work.tile([128, 2, 432], F32, tag="t2")
                nc.vector.tensor_tensor(out=t2[:, :, 0:w], in0=t4[:, 0:2, 0:w],
                                        in1=t4[:, 2:4, 0:w], op=AL.max)
                mx = work.tile([128, 432], F32, tag="mx")
                nc.vector.tensor_tensor(out=mx[:, 0:w], in0=t2[:, 0, 0:w],
                                        in1=t2[:, 1, 0:w], op=AL.max)
                nc.vector.tensor_tensor(out=mx[:, 0:w], in0=mx[:, 0:w],
                                        in1=t9[:, 8, 0:w], op=AL.max)
                mxb = work.tile([128, 432], F32, tag="mxb")
                nc.gpsimd.partition_all_reduce(mxb[:, 0:w], mx[:, 0:w],
                                               channels=128,
                                               reduce_op=bass_isa.ReduceOp.max)
                mview = bass.AP(tensor=mxb.tensor, offset=mxb.offset,
                                ap=[mxb.ap[0], [0, J], [1, w]])
                nc.vector.tensor_tensor(out=Lt[:, :, 0:w], in0=Lt[:, :, 0:w],
                                        in1=mview, op=AL.subtract)
                # E = exp(u) -> bf16 overlay
                nc.scalar.activation(Ebig[:, :, d0:d0 + w], Lt[:, :, 0:w], AF.Exp)
                # Z = ones^T E (before mask-zeroing)
                zp = psz.tile([1, 432], F32, tag="zp")
                for j in range(J):
                    nc.tensor.matmul(zp[:, 0:w], ones16, Ebig[:, j, d0:d0 + w],
                                     start=(j == 0), stop=(j == J - 1))
                nc.scalar.activation(Zrow[:, d0:d0 + w], zp[:, 0:w], AF.Copy)
                # zero masked rows for recon (pool engine)
                for j in range(J):
                    nc.gpsimd.tensor_scalar_mul(Ebig[:, j, d0:d0 + w],
                                                Ebig[:, j, d0:d0 + w],
                                                mbint[:, j:j + 1])
        # E gates (phantom att cols)
        nc.gpsimd.tensor_scalar_mul(Ebig[:, :, 0:G], Ebig[:, :, 0:G], gatet[:, 0:1])
        nc.gpsimd.tensor_scalar_mul(Ebig[:, :, NA - G:NA], Ebig[:, :, NA - G:NA],
                                    gatet[:, 1:2])
        if debug:
            nc.sync.dma_start(out=dE_d[:, :, :], in_=Ebig)
            nc.sync.dma_start(out=dZ_d[:, :], in_=Zrow)

        # recipZ = 0.25 / Z broadcast
        rzrow = consts.tile([1, NA], F32, tag="rzrow")
        nc.vector.reciprocal(rzrow, Zrow)
        nc.vector.tensor_scalar_mul(rzrow, rzrow, 0.25)
        rzb = consts.tile([128, NA], F32, tag="rzb")
        nc.gpsimd.partition_broadcast(rzb, rzrow)

        # ---------------- recon + interleave ----------------
        out_acc = big.tile([128, 48, 96], F32, tag="U1")
        nc.vector.memset(out_acc, 0.0)
        ky_pis = {0: (4, 28), 1: (3, 27), 2: (3, 27), 3: (2, 26)}
        kx_us = {0: (1, 48), 1: (0, 48), 2: (0, 48), 3: (0, 47)}
        rchunk_off = [0, 432, 864]
        RCH = (432, 432, 384)
        with tc.tile_pool(name="psg", bufs=4, space="PSUM") as psg:
            for tap in range(16):
                ky, kx = tap // 4, tap % 4
                rawtile = rawp.tile([128, J * 128], BF16, tag="rawtile")
                nc.sync.dma_start(out=rawtile, in_=rawt_d[tap, :, :])
                plo, phi = ky_pis[ky]
                ulo, uhi = kx_us[kx]
                for ri, rw in enumerate(RCH):
                    r0 = rchunk_off[ri]
                    cplo = 2 + r0 // G
                    cphi = 2 + (r0 + rw) // G
                    a = max(plo, cplo); bnd = min(phi, cphi)
                    if a >= bnd:
                        continue
                    # trim matmul/scale to used pi blocks
                    A = (a - 2) * G - r0
                    B2 = (bnd - 2) * G - r0
                    gp = psg.tile([128, 432], F32, tag="gp")
                    for j in range(J):
                        nc.tensor.matmul(gp[:, A:B2], rawtile[:, j * 128:(j + 1) * 128],
                                         Ebig[:, j, r0 + A:r0 + B2],
                                         start=(j == 0), stop=(j == J - 1))
                    gs = gsp.tile([128, 432], BF16, tag="gs")
                    nc.vector.tensor_tensor(out=gs[:, A:B2], in0=gp[:, A:B2],
                                            in1=rzb[:, r0 + A:r0 + B2], op=AL.mult)
                    npi = bnd - a
                    nu = uhi - ulo
                    goff = (a - 2) * G + ulo - r0
                    gview = bass.AP(tensor=gs.tensor, offset=gs.offset + goff,
                                    ap=[gs.ap[0], [G, npi], [1, nu]])
                    yl0 = 2 * (a - 3) + ky - 1
                    xl0 = 2 * ulo + kx - 1
                    oview = bass.AP(tensor=out_acc.tensor,
                                    offset=out_acc.offset + yl0 * 96 + xl0,
                                    ap=[out_acc.ap[0], [192, npi], [2, nu]])
                    eng = nc.gpsimd if (tap % 2 == 0) else nc.vector
                    eng.tensor_tensor(out=oview, in0=oview, in1=gview, op=AL.add)
        nc.sync.dma_start(out=out_d[:, :, :], in_=out_acc)
    nc.finalize()
    return nc


# ======================= host side =======================

def make_shift_mats():
    m = np.zeros((10, 128, 128), np.float16)
    m[UP1] = np.eye(128, k=-1)     # [k,m]: k=m+1
    m[DN1] = np.eye(128, k=1)      # k=m-1
    m[CUP1, 0, 127] = 1.0
    m[CDN1, 127, 0] = 1.0
    m[UP48] = np.eye(128, k=-48)   # k=m+48 (m<=79)
    m[DN48] = np.eye(128, k=48)    # k=m-48 (m>=48)
    m[CUP48] = np.eye(128, k=80)   # k=m-80 (m in 80..127)
    m[CDN48] = np.eye(128, k=-80)  # k=m+80 (m in 0..47)
    p79 = np.eye(128, k=79)        # m=k+79
    p79[0, :] = 0.0
    p79[48:, :] = 0.0              # keep k in [1,47]
    m[P79] = p79
    pm79 = np.eye(128, k=-79)      # m=k-79
    pm79[79, :] = 0.0
    pm79[127, :] = 0.0             # keep k in [80,126]
    m[PM79] = pm79
    return m


def prep_core_inputs(f, b, mask):
    """Full inputs -> list of 8 in_map dicts (core = 2*s + q)."""
    B = f.shape[0]
    ms = np.pad(mask[0][:, ::8, ::8][0], 1)
    w = np.lib.stride_tricks.sliding_window_view(ms, (3, 3))
    mm = (w.sum((2, 3)) == 0).astype(np.float32).reshape(LB)
    s10 = np.ascontiguousarray((10.0 * mm).reshape(J, 128).T)
    mbin = np.ascontiguousarray(mm.reshape(J, 128).T)
    shm = make_shift_mats()
    in_maps = []
    for s in range(B):
        fs = f[s][:, ::2, ::2]
        bs = b[s][:, ::2, ::2]
        fsp = np.pad(fs, ((0, 0), (1, 1), (1, 1)))
        bsp = np.pad(bs, ((0, 0), (1, 1), (1, 1)))
        bhwc = np.pad(b[s], ((0, 0), (1, 1), (1, 1))).transpose(1, 2, 0)
        wt = np.empty((KT, C, LB), np.float32)
        for o in range(KT):
            dy, dx = o // 3, o % 3
            wt[o] = bsp[:, dy:dy + G, dx:dx + G].reshape(C, LB)
        # rdent: 1/sqrt(patch sum of squares + 1152*eps), [128, J]
        ssq = np.zeros((G + 2, G + 2), np.float32)
        ssq[1:G + 1, 1:G + 1] = (bs * bs).sum(0)
        sw = np.lib.stride_tricks.sliding_window_view(ssq, (3, 3)).sum((2, 3))
        rd = 1.0 / np.sqrt(sw.reshape(LB) + ESC_BIAS)
        rdent = np.ascontiguousarray(rd.reshape(J, 128).T).astype(np.float32)
        # wt layout [J, C, KT*128]: [j, c, o*128+m] = wt[o, c, j*128+m]
        wt_blocks = np.ascontiguousarray(
            wt.reshape(KT, C, J, 128).transpose(2, 1, 0, 3).reshape(J, C, KT * 128)
        ).astype(np.float16)
        iy, ix = np.divmod(np.arange(LB), G)
        rawt = np.empty((16, LB, C), np.float32)
        for ky in range(4):
            for kx in range(4):
                rawt[ky * 4 + kx] = bhwc[2 * iy + ky, 2 * ix + kx, :]
        # rawt layout [16, 128, J*128]: [tap, p, j*128+c] = rawt[tap, j*128+p, c]
        rawt_blocks = np.ascontiguousarray(
            rawt.reshape(16, J, 128, C).transpose(0, 2, 1, 3).reshape(16, 128, J * C)
        ).astype(ml_dtypes.bfloat16)
        for q in (0, 1):
            ts_ = np.arange(WINP) - 3 + 24 * q
            fcols = np.zeros((KT, C, NCOL), np.float32)
            valid = (ts_ >= 0) & (ts_ < G)
            for o in range(KT):
                dy, dx = o // 3, o % 3
                block = fsp[:, (ts_ + dy).clip(0, G + 1), :][:, :, dx:dx + G]
                block = block * valid[None, :, None]
                fcols[o, :, :WIN] = block.reshape(C, WIN)
                if q == 1:
                    fcols[o, :, FT0:FT0 + 96] = fsp[:, dy:dy + 2, dx:dx + G].reshape(C, 96)
                else:
                    fcols[o, :, FB0:FB0 + 96] = fsp[:, 46 + dy:48 + dy, dx:dx + G].reshape(C, 96)
            gate = np.zeros((128, 2), np.float32)
            gate[:, 0] = 0.0 if q == 0 else 1.0
            gate[:, 1] = 1.0 if q == 0 else 0.0
            in_maps.append(dict(
                fp=fcols.astype(np.float16),
                wt=wt_blocks,
                rawt=rawt_blocks,
                shm=shm,
                rdent=rdent,
                s10=s10, mbin=mbin, gate=gate,
            ))
    return in_maps


def assemble(results, B=4):
    out = np.zeros((B, C, 96, 96), np.float32)
    for s in range(B):
        for q in (0, 1):
            out[s, :, 48 * q:48 * q + 48, :] = results[2 * s + q]["out"]
    return out


# ======================= self-contained runner =======================
# kernel(**inputs) entry point: full inputs in, full output out.
_NC_CACHE = {}
last_exec_time_ns = None
last_result = None


def kernel(f, b, mask):
    global last_exec_time_ns, last_result
    import os
    from concourse.bass_utils import run_bass_kernel_spmd
    f = np.ascontiguousarray(np.asarray(f, dtype=np.float32))
    b = np.ascontiguousarray(np.asarray(b, dtype=np.float32))
    mask = np.ascontiguousarray(np.asarray(mask, dtype=np.float32))
    in_maps = prep_core_inputs(f, b, mask)
    if "nc" not in _NC_CACHE:
        _NC_CACHE["nc"] = build(debug=False)
    nc = _NC_CACHE["nc"]
    trace = bool(os.environ.get("BASS_TRACE"))
    res = run_bass_kernel_spmd(nc, in_maps, core_ids=list(range(8)), trace=trace)
    last_result = res
    last_exec_time_ns = res.exec_time_ns
    return assemble([res.results[i] for i in range(8)], B=f.shape[0])


# revision 16
# speedup vs baseline: 3.2528x; 1.6240x over previous
"""Bass kernel builder + host prep for nn_ContextualAttention on 8 trn2 cores.

Sharding: core = 2*s + q (s = sample 0..3, q = lf-half 0..1).
Window: 30 grid-row positions pi in [0,30), true row t(pi) = pi - 3 + 24q.
Score cols: [0,1440) window, [1440,1536) far_top, [1536,1632) far_bot.
Consumed (softmax/recon) cols: window pi in [2,28) -> global [96,1344), NA=1248.

v2: no SBUF->SBUF shift DMAs. All partition shifts (diagonal fuse pass-1/2,
far corrections) are tensor-engine permutation matmuls into PSUM; hosts ships
shift matrices. slab1 holds Sn (f16, 1632 cols, E bf16 overlays cols [0,1248)
after pass-1); slab2 holds S1 for exactly the consumed sources: cols [0,1344)
= global [48,1392), [1344,1392) = far_top [1441,1488), [1392,1440) = far_bot
[1584,1631). rden moved to host (rdent input). wt/rawt layouts are dense per
partition.
"""
import numpy as np
import ml_dtypes
import contextlib
import concourse.bass as bass
from concourse import bacc, bass_isa
import concourse.tile as tile
from concourse import mybir

F16 = mybir.dt.float16
F32 = mybir.dt.float32
BF16 = mybir.dt.bfloat16
AL = mybir.AluOpType
AF = mybir.ActivationFunctionType

G = 48
J = 18
KT = 9
LB = 2304
C = 128
WINP = 30
WIN = WINP * G          # 1440
FT0 = WIN               # 1440
FB0 = WIN + 96          # 1536
NCOL = WIN + 192        # 1632
NA = 26 * G             # 1248
ESC_BIAS = 1152 * 1e-4

# shift-matrix indices in shm input
UP1, DN1, CUP1, CDN1, UP48, DN48, CUP48, CDN48, P79, PM79 = range(10)

# scores GEMM chunks over slab1 cols
SCORE_CHUNKS = ((0, 512), (512, 512), (1024, 512), (1536, 96))
# pass-1 chunks over slab2 cols (far sub-blocks only need 47 cols each;
# slab2 cols 1391 and 1439 stay unwritten/unread). Far chunks first so
# pass-2 chunk 0 (which reads the far-bot cols) unblocks earliest.
P1_CHUNKS = ((1344, 47), (1392, 47), (0, 512), (512, 512), (1024, 320))
# pass-2 chunks: (global col start, width); local d0 = g0 - 96
P2_CHUNKS = ((96, 432), (528, 432), (960, 384))
# far correction target global col ranges (47 wide each)
B_LO, B_HI = 26 * G, 26 * G + G - 1        # ftP/addC targets (chunk 2)
BP_LO, BP_HI = 3 * G + 1, 3 * G + G        # fbM/addCp targets (chunk 0)


def s2g(c):
    """slab2 col -> slab1 (global score) col."""
    if c < 1344:
        return c + 48
    if c < 1392:
        return c - 1344 + 1441
    return c - 1392 + 1584


def build(debug=False):
    nc = bacc.Bacc()
    fp_d = nc.dram_tensor("fp", [KT, 128, NCOL], F16, kind="ExternalInput")
    wt_d = nc.dram_tensor("wt", [J, 128, KT * 128], F16, kind="ExternalInput")
    rawt_d = nc.dram_tensor("rawt", [16, 128, J * 128], BF16, kind="ExternalInput")
    shm_d = nc.dram_tensor("shm", [10, 128, 128], F16, kind="ExternalInput")
    rdent_d = nc.dram_tensor("rdent", [128, J], F32, kind="ExternalInput")
    s10_d = nc.dram_tensor("s10", [128, J], F32, kind="ExternalInput")
    gcol_d = nc.dram_tensor("gcol", [1, NA], F32, kind="ExternalInput")
    gate_d = nc.dram_tensor("gate", [128, 2], F32, kind="ExternalInput")
    out_d = nc.dram_tensor("out", [128, 48, 96], F32, kind="ExternalOutput")
    if debug:
        dSn_d = nc.dram_tensor("dSn", [128, J, NCOL], F16, kind="ExternalOutput")
        dS1_d = nc.dram_tensor("dS1", [128, J, 1440], F16, kind="ExternalOutput")
        dE_d = nc.dram_tensor("dE", [128, J, NA], BF16, kind="ExternalOutput")
        dZ_d = nc.dram_tensor("dZ", [1, NA], F32, kind="ExternalOutput")

    with tile.TileContext(nc) as tc, contextlib.ExitStack() as ctx:
        consts = ctx.enter_context(tc.tile_pool(name="consts", bufs=1))
        wtp = ctx.enter_context(tc.tile_pool(name="wtp", bufs=2))
        big = ctx.enter_context(tc.tile_pool(name="big", bufs=1))
        lpool = ctx.enter_context(tc.tile_pool(name="lpool", bufs=1))
        work = ctx.enter_context(tc.tile_pool(name="work", bufs=1))
        rawp = ctx.enter_context(tc.tile_pool(name="rawp", bufs=2))
        gsp = ctx.enter_context(tc.tile_pool(name="gsp", bufs=2))

        # ---------------- consts / small inputs ----------------
        s10t = consts.tile([128, J], F32, tag="s10t")
        nc.sync.dma_start(out=s10t, in_=s10_d[:, :])
        gcolt = consts.tile([1, NA], F32, tag="gcolt")
        nc.sync.dma_start(out=gcolt, in_=gcol_d[:, :])
        gatet = consts.tile([128, 2], F32, tag="gatet")
        nc.sync.dma_start(out=gatet, in_=gate_d[:, :])
        rdent = consts.tile([128, J], F32, tag="rdent")
        nc.sync.dma_start(out=rdent, in_=rdent_d[:, :])
        shmt = consts.tile([128, 10, 128], F16, tag="shmt")
        for i in range(10):
            nc.sync.dma_start(out=shmt[:, i, :], in_=shm_d[i, :, :])
        ones16 = consts.tile([128, 1], BF16, tag="ones16")
        nc.vector.memset(ones16, 1.0)

        def shmat(i):
            return shmt[:, i, :]

        # ---------------- scores GEMM -> slab1 (Sn, f16) ----------------
        fpt = big.tile([128, KT, NCOL], F16, tag="U1")
        for o in range(KT):
            nc.sync.dma_start(out=fpt[:, o, :], in_=fp_d[o, :, :])
        slab1 = big.tile([128, J, NCOL], F16, tag="slab1")
        with tc.tile_pool(name="psc", bufs=4, space="PSUM") as psc:
            for j in range(J):
                wtj = wtp.tile([128, KT * 128], F16, tag="wtj")
                nc.sync.dma_start(out=wtj, in_=wt_d[j, :, :])
                for c0, w in SCORE_CHUNKS:
                    ps = psc.tile([128, 512], F32, tag="sps")
                    for o in range(KT):
                        nc.tensor.matmul(ps[:, 0:w], wtj[:, o * 128:(o + 1) * 128],
                                         fpt[:, o, c0:c0 + w],
                                         start=(o == 0), stop=(o == KT - 1))
                    nc.scalar.activation(slab1[:, j, c0:c0 + w], ps[:, 0:w],
                                         AF.Copy, scale=rdent[:, j:j + 1])
        if debug:
            nc.sync.dma_start(out=dSn_d[:, :, :], in_=slab1)

        # ---------------- pass-1: S1 = Sn + diag(+1) + diag(-1) -> slab2 ----
        # slab2 shares the U1 slot with fpt (dead after scores GEMM) and
        # out_acc (recon starts after pass-2 ends)
        slab2 = big.tile([128, J, 1440], F16, tag="U1")
        with tc.tile_pool(name="psp1", bufs=4, space="PSUM") as psp1:
            for c0, w in P1_CHUNKS:
                g0 = s2g(c0)
                for j in range(J):
                    ps = psp1.tile([128, 512], F32, tag="p1ps")
                    nc.tensor.matmul(ps[:, 0:w], shmat(UP1),
                                     slab1[:, j, g0 + 1:g0 + 1 + w],
                                     start=True, stop=False)
                    if j < J - 1:
                        nc.tensor.matmul(ps[:, 0:w], shmat(CUP1),
                                         slab1[:, j + 1, g0 + 1:g0 + 1 + w],
                                         start=False, stop=False)
                    if j > 0:
                        nc.tensor.matmul(ps[:, 0:w], shmat(CDN1),
                                         slab1[:, j - 1, g0 - 1:g0 - 1 + w],
                                         start=False, stop=False)
                    nc.tensor.matmul(ps[:, 0:w], shmat(DN1),
                                     slab1[:, j, g0 - 1:g0 - 1 + w],
                                     start=False, stop=True)
                    nc.vector.tensor_tensor(out=slab2[:, j, c0:c0 + w],
                                            in0=ps[:, 0:w],
                                            in1=slab1[:, j, g0:g0 + w], op=AL.add)
        # gates: zero pi=2 block (q=0) / pi=27 block (q=1); slab2 cols = g-48
        nc.vector.tensor_scalar_mul(slab2[:, :, 48:96], slab2[:, :, 48:96],
                                    gatet[:, 0:1])
        nc.vector.tensor_scalar_mul(slab2[:, :, 1248:1296], slab2[:, :, 1248:1296],
                                    gatet[:, 1:2])
        if debug:
            nc.sync.dma_start(out=dS1_d[:, :, :], in_=slab2)

        # E overlays slab1 cols [0, NA) as bf16 (Sn dead after pass-1)
        Ebig = slab1[:, :, 0:NA].bitcast(BF16)
        Zrow = consts.tile([1, NA], F32, tag="Zrow")

        # ---------------- pass-2 + softmax per chunk ----------------
        with tc.tile_pool(name="psp2", bufs=4, space="PSUM") as psp2, \
             tc.tile_pool(name="psz", bufs=2, space="PSUM") as psz:
            for ci, (g0, w) in enumerate(P2_CHUNKS):
                d0 = g0 - 96
                c0 = g0 - 48                       # slab2 col of g0
                Lt = lpool.tile([128, J, 432], F32, tag="Lt")
                for j in range(J):
                    ps = psp2.tile([128, 432], F32, tag="p2ps")
                    nc.tensor.matmul(ps[:, 0:w], shmat(UP48),
                                     slab2[:, j, c0 + 48:c0 + 48 + w],
                                     start=True, stop=False)
                    if j < J - 1:
                        nc.tensor.matmul(ps[:, 0:w], shmat(CUP48),
                                         slab2[:, j + 1, c0 + 48:c0 + 48 + w],
                                         start=False, stop=False)
                    if j > 0:
                        nc.tensor.matmul(ps[:, 0:w], shmat(CDN48),
                                         slab2[:, j - 1, c0 - 48:c0 - 48 + w],
                                         start=False, stop=False)
                    # row-wrap terms (by=47 up-wrap at j=17, by=0 dn-wrap at j=0)
                    if j == J - 1:
                        nc.tensor.matmul(ps[:, 0:w], shmat(P79),
                                         slab2[:, 0, c0 + 48:c0 + 48 + w],
                                         start=False, stop=False)
                    if j == 0:
                        nc.tensor.matmul(ps[:, 0:w], shmat(PM79),
                                         slab2[:, J - 1, c0 - 48:c0 - 48 + w],
                                         start=False, stop=False)
                    if ci == 2:
                        # B targets [B_LO, B_HI): psum cols, ft sources
                        a, b = B_LO - g0, B_HI - g0
                        nw = b - a
                        nc.tensor.matmul(ps[:, a:b], shmat(UP48),
                                         slab2[:, j, 1344:1344 + nw],
                                         start=False, stop=False)
                        if j < J - 1:
                            nc.tensor.matmul(ps[:, a:b], shmat(CUP48),
                                             slab2[:, j + 1, 1344:1344 + nw],
                                             start=False, stop=False)
                        if j == J - 1:
                            nc.tensor.matmul(ps[:, a:b], shmat(P79),
                                             slab2[:, 0, 1344:1344 + nw],
                                             start=False, stop=False)
                    if ci == 0:
                        # B' targets [BP_LO, BP_HI): fb sources
                        a, b = BP_LO - g0, BP_HI - g0
                        nw = b - a
                        nc.tensor.matmul(ps[:, a:b], shmat(DN48),
                                         slab2[:, j, 1392:1392 + nw],
                                         start=False, stop=False)
                        if j > 0:
                            nc.tensor.matmul(ps[:, a:b], shmat(CDN48),
                                             slab2[:, j - 1, 1392:1392 + nw],
                                             start=False, stop=False)
                        if j == 0:
                            nc.tensor.matmul(ps[:, a:b], shmat(PM79),
                                             slab2[:, J - 1, 1392:1392 + nw],
                                             start=False, stop=False)
                    nc.tensor.matmul(ps[:, 0:w], shmat(DN48),
                                     slab2[:, j, c0 - 48:c0 - 48 + w],
                                     start=False, stop=True)
                    # S2 = psum + S1, then L = S2 * s10 (scalar engine)
                    nc.vector.tensor_tensor(out=Lt[:, j, 0:w], in0=ps[:, 0:w],
                                            in1=slab2[:, j, c0:c0 + w], op=AL.add)
                    nc.scalar.activation(Lt[:, j, 0:w], Lt[:, j, 0:w],
                                         AF.Copy, scale=s10t[:, j:j + 1])
                # max over lb: tree over j, then across partitions
                t9 = work.tile([128, 9, 432], F32, tag="shA")
                nc.vector.tensor_tensor(out=t9[:, :, 0:w], in0=Lt[:, 0:9, 0:w],
                                        in1=Lt[:, 9:18, 0:w], op=AL.max)
                t4 = work.tile([128, 4, 432], F32, tag="shB")
                nc.vector.tensor_tensor(out=t4[:, :, 0:w], in0=t9[:, 0:4, 0:w],
                                        in1=t9[:, 4:8, 0:w], op=AL.max)
                t2 = work.tile([128, 2, 432], F32, tag="t2")
                nc.vector.tensor_tensor(out=t2[:, :, 0:w], in0=t4[:, 0:2, 0:w],
                                        in1=t4[:, 2:4, 0:w], op=AL.max)
                mx = work.tile([128, 432], F32, tag="mx")
                nc.vector.tensor_tensor(out=mx[:, 0:w], in0=t2[:, 0, 0:w],
                                        in1=t2[:, 1, 0:w], op=AL.max)
                nc.vector.tensor_tensor(out=mx[:, 0:w], in0=mx[:, 0:w],
                                        in1=t9[:, 8, 0:w], op=AL.max)
                mxb = work.tile([128, 432], F32, tag="mxb")
                nc.gpsimd.partition_all_reduce(mxb[:, 0:w], mx[:, 0:w],
                                               channels=128,
                                               reduce_op=bass_isa.ReduceOp.max)
                mview = bass.AP(tensor=mxb.tensor, offset=mxb.offset,
                                ap=[mxb.ap[0], [0, J], [1, w]])
                nc.vector.tensor_tensor(out=Lt[:, :, 0:w], in0=Lt[:, :, 0:w],
                                        in1=mview, op=AL.subtract)
                # E = exp(u) -> bf16 overlay
                nc.scalar.activation(Ebig[:, :, d0:d0 + w], Lt[:, :, 0:w], AF.Exp)
                # Z = ones^T E (before mask-zeroing)
                zp = psz.tile([1, 432], F32, tag="zp")
                for j in range(J):
                    nc.tensor.matmul(zp[:, 0:w], ones16, Ebig[:, j, d0:d0 + w],
                                     start=(j == 0), stop=(j == J - 1))
                nc.scalar.activation(Zrow[:, d0:d0 + w], zp[:, 0:w], AF.Copy)
        # masked rows are folded into rawt on the host (mm-scaled); phantom
        # att cols are folded into the rz row via gcol — E needs no zeroing
        if debug:
            nc.sync.dma_start(out=dE_d[:, :, :], in_=Ebig)
            nc.sync.dma_start(out=dZ_d[:, :], in_=Zrow)

        # recipZ = 0.25 * gcol / Z broadcast
        rzrow = consts.tile([1, NA], F32, tag="rzrow")
        nc.vector.reciprocal(rzrow, Zrow)
        nc.vector.tensor_tensor(out=rzrow, in0=rzrow, in1=gcolt, op=AL.mult)
        rzb = consts.tile([128, NA], F32, tag="rzb")
        nc.gpsimd.partition_broadcast(rzb, rzrow)

        # ---------------- recon + interleave ----------------
        out_acc = big.tile([128, 48, 96], F32, tag="U1")
        nc.vector.memset(out_acc, 0.0)
        ky_pis = {0: (4, 28), 1: (3, 27), 2: (3, 27), 3: (2, 26)}
        kx_us = {0: (1, 48), 1: (0, 48), 2: (0, 48), 3: (0, 47)}
        rchunk_off = [0, 432, 864]
        RCH = (432, 432, 384)
        with tc.tile_pool(name="psg", bufs=4, space="PSUM") as psg:
            for tap in range(16):
                ky, kx = tap // 4, tap % 4
                rawtile = rawp.tile([128, J * 128], BF16, tag="rawtile")
                nc.sync.dma_start(out=rawtile, in_=rawt_d[tap, :, :])
                plo, phi = ky_pis[ky]
                ulo, uhi = kx_us[kx]
                for ri, rw in enumerate(RCH):
                    r0 = rchunk_off[ri]
                    cplo = 2 + r0 // G
                    cphi = 2 + (r0 + rw) // G
                    a = max(plo, cplo); bnd = min(phi, cphi)
                    if a >= bnd:
                        continue
                    # trim matmul/scale to used pi blocks
                    A = (a - 2) * G - r0
                    B2 = (bnd - 2) * G - r0
                    gp = psg.tile([128, 432], F32, tag="gp")
                    for j in range(J):
                        nc.tensor.matmul(gp[:, A:B2], rawtile[:, j * 128:(j + 1) * 128],
                                         Ebig[:, j, r0 + A:r0 + B2],
                                         start=(j == 0), stop=(j == J - 1))
                    gs = gsp.tile([128, 432], BF16, tag="gs")
                    nc.vector.tensor_tensor(out=gs[:, A:B2], in0=gp[:, A:B2],
                                            in1=rzb[:, r0 + A:r0 + B2], op=AL.mult)
                    npi = bnd - a
                    nu = uhi - ulo
                    goff = (a - 2) * G + ulo - r0
                    gview = bass.AP(tensor=gs.tensor, offset=gs.offset + goff,
                                    ap=[gs.ap[0], [G, npi], [1, nu]])
                    yl0 = 2 * (a - 3) + ky - 1
                    xl0 = 2 * ulo + kx - 1
                    oview = bass.AP(tensor=out_acc.tensor,
                                    offset=out_acc.offset + yl0 * 96 + xl0,
                                    ap=[out_acc.ap[0], [192, npi], [2, nu]])
                    eng = nc.gpsimd if (tap % 2 == 0) else nc.vector
                    eng.tensor_tensor(out=oview, in0=oview, in1=gview, op=AL.add)
        nc.sync.dma_start(out=out_d[:, :, :], in_=out_acc)
    nc.finalize()
    return nc


# ======================= host side =======================

def make_shift_mats():
    m = np.zeros((10, 128, 128), np.float16)
    m[UP1] = np.eye(128, k=-1)     # [k,m]: k=m+1
    m[DN1] = np.eye(128, k=1)      # k=m-1
    m[CUP1, 0, 127] = 1.0
    m[CDN1, 127, 0] = 1.0
    m[UP48] = np.eye(128, k=-48)   # k=m+48 (m<=79)
    m[DN48] = np.eye(128, k=48)    # k=m-48 (m>=48)
    m[CUP48] = np.eye(128, k=80)   # k=m-80 (m in 80..127)
    m[CDN48] = np.eye(128, k=-80)  # k=m+80 (m in 0..47)
    p79 = np.eye(128, k=79)        # m=k+79
    p79[0, :] = 0.0
    p79[48:, :] = 0.0              # keep k in [1,47]
    m[P79] = p79
    pm79 = np.eye(128, k=-79)      # m=k-79
    pm79[79, :] = 0.0
    pm79[127, :] = 0.0             # keep k in [80,126]
    m[PM79] = pm79
    return m


def prep_core_inputs(f, b, mask):
    """Full inputs -> list of 8 in_map dicts (core = 2*s + q)."""
    B = f.shape[0]
    ms = np.pad(mask[0][:, ::8, ::8][0], 1)
    w = np.lib.stride_tricks.sliding_window_view(ms, (3, 3))
    mm = (w.sum((2, 3)) == 0).astype(np.float32).reshape(LB)
    s10 = np.ascontiguousarray((10.0 * mm).reshape(J, 128).T)
    mbin = np.ascontiguousarray(mm.reshape(J, 128).T)
    shm = make_shift_mats()
    in_maps = []
    for s in range(B):
        fs = f[s][:, ::2, ::2]
        bs = b[s][:, ::2, ::2]
        fsp = np.pad(fs, ((0, 0), (1, 1), (1, 1)))
        bsp = np.pad(bs, ((0, 0), (1, 1), (1, 1)))
        bhwc = np.pad(b[s], ((0, 0), (1, 1), (1, 1))).transpose(1, 2, 0)
        wt = np.empty((KT, C, LB), np.float32)
        for o in range(KT):
            dy, dx = o // 3, o % 3
            wt[o] = bsp[:, dy:dy + G, dx:dx + G].reshape(C, LB)
        # rdent: 1/sqrt(patch sum of squares + 1152*eps), [128, J]
        ssq = np.zeros((G + 2, G + 2), np.float32)
        ssq[1:G + 1, 1:G + 1] = (bs * bs).sum(0)
        sw = np.lib.stride_tricks.sliding_window_view(ssq, (3, 3)).sum((2, 3))
        rd = 1.0 / np.sqrt(sw.reshape(LB) + ESC_BIAS)
        rdent = np.ascontiguousarray(rd.reshape(J, 128).T).astype(np.float32)
        # wt layout [J, C, KT*128]: [j, c, o*128+m] = wt[o, c, j*128+m]
        wt_blocks = np.ascontiguousarray(
            wt.reshape(KT, C, J, 128).transpose(2, 1, 0, 3).reshape(J, C, KT * 128)
        ).astype(np.float16)
        iy, ix = np.divmod(np.arange(LB), G)
        rawt = np.empty((16, LB, C), np.float32)
        for ky in range(4):
            for kx in range(4):
                rawt[ky * 4 + kx] = bhwc[2 * iy + ky, 2 * ix + kx, :]
        rawt *= mm[None, :, None]      # masked rows contribute 0 to recon
        # rawt layout [16, 128, J*128]: [tap, p, j*128+c] = rawt[tap, j*128+p, c]
        rawt_blocks = np.ascontiguousarray(
            rawt.reshape(16, J, 128, C).transpose(0, 2, 1, 3).reshape(16, 128, J * C)
        ).astype(ml_dtypes.bfloat16)
        for q in (0, 1):
            ts_ = np.arange(WINP) - 3 + 24 * q
            fcols = np.zeros((KT, C, NCOL), np.float32)
            valid = (ts_ >= 0) & (ts_ < G)
            for o in range(KT):
                dy, dx = o // 3, o % 3
                block = fsp[:, (ts_ + dy).clip(0, G + 1), :][:, :, dx:dx + G]
                block = block * valid[None, :, None]
                fcols[o, :, :WIN] = block.reshape(C, WIN)
                if q == 1:
                    fcols[o, :, FT0:FT0 + 96] = fsp[:, dy:dy + 2, dx:dx + G].reshape(C, 96)
                else:
                    fcols[o, :, FB0:FB0 + 96] = fsp[:, 46 + dy:48 + dy, dx:dx + G].reshape(C, 96)
            gate = np.zeros((128, 2), np.float32)
            gate[:, 0] = 0.0 if q == 0 else 1.0
            gate[:, 1] = 1.0 if q == 0 else 0.0
            gcol = np.full((1, NA), 0.25, np.float32)   # 0.25 recon scale folded in
            if q == 0:
                gcol[0, 0:G] = 0.0
            else:
                gcol[0, NA - G:NA] = 0.0
            in_maps.append(dict(
                fp=fcols.astype(np.float16),
                wt=wt_blocks,
                rawt=rawt_blocks,
                shm=shm,
                rdent=rdent,
                s10=s10, gcol=gcol, gate=gate,
            ))
    return in_maps


def assemble(results, B=4):
    out = np.zeros((B, C, 96, 96), np.float32)
    for s in range(B):
        for q in (0, 1):
            out[s, :, 48 * q:48 * q + 48, :] = results[2 * s + q]["out"]
    return out


# ======================= self-contained runner =======================
# kernel(**inputs) entry point: full inputs in, full output out.
_NC_CACHE = {}
last_exec_time_ns = None
last_result = None


def kernel(f, b, mask):
    global last_exec_time_ns, last_result
    import os
    from concourse.bass_utils import run_bass_kernel_spmd
    f = np.ascontiguousarray(np.asarray(f, dtype=np.float32))
    b = np.ascontiguousarray(np.asarray(b, dtype=np.float32))
    mask = np.ascontiguousarray(np.asarray(mask, dtype=np.float32))
    in_maps = prep_core_inputs(f, b, mask)
    if "nc" not in _NC_CACHE:
        _NC_CACHE["nc"] = build(debug=False)
    nc = _NC_CACHE["nc"]
    trace = bool(os.environ.get("BASS_TRACE"))
    res = run_bass_kernel_spmd(nc, in_maps, core_ids=list(range(8)), trace=trace)
    last_result = res
    last_exec_time_ns = res.exec_time_ns
    return assemble([res.results[i] for i in range(8)], B=f.shape[0])
